# revision 4
# baseline (speedup 1.0000x reference)
"""Multi-head attention forward (B=4, N=2048, C=512, H=8, D=64) on 8 TRN2 cores.

Sharding: core = 2*b + g  (b = batch 0..3, g = head-group 0..1, 4 heads each).
Each core computes a partial projection output for its batch from its 4 heads;
the host sums the two group partials and adds proj bias.

Per-core dataflow (all layouts chosen so no on-device transposes are needed):
  xT [C, N] resident in SBUF.
  qk^T = Wqk^T.T @ x^T         -> [512(o), 2048(n)]  (o on partitions)
  v    = x^T.T @ Wv^T          -> [2048(m), 256(dv)] (m on partitions) + ones col
  per head pair (parity packs the 64-wide contraction into both PE row groups):
    S^T chunk [128(m), 2, 512(n)] = k^T.T @ q^T   (K=64 row-tiled matmuls)
    P^T = exp(S^T * 0.125)  on ACT engine, PSUM -> SBUF, one instr per pair
    outT'[65, n] += V_aug.T @ P^T   (row 64 accumulates softmax denominator)
  normalize: recip of row 64, K=1 matmul broadcasts it across partitions,
  multiply -> outT_norm [64(d), 4(h), 2048(n)].
  y_partial[n, co] = sum_h outT_norm_h.T @ projT_h  (K=64 accumulated in PSUM)

float32r tensors hold ordinary fp32 bits; the tag lets the PE run matmuls at
1 row/cycle (vs 4 for strict fp32).
"""

from contextlib import ExitStack

import numpy as np

import concourse.bass as bass
import concourse.mybir as mybir
import concourse.tile as tile
from concourse import bacc
from concourse.bass_utils import run_bass_kernel_spmd

F32 = mybir.dt.float32
F32R = mybir.dt.float32r

B, N, C = 4, 2048, 512
H_PER_CORE = 4
D = 64
NT512 = N // 512          # 4 chunks of 512 along n
MT = N // 128             # 16 m tiles
EXP_SCALE = 1.0 / np.sqrt(D)


def build_nc():
    nc = bacc.Bacc(
        "TRN2",
        target_bir_lowering=False,
        debug=False,
        enable_asserts=False,
        num_devices=8,
    )

    xt_d = nc.dram_tensor("xt", [C, N], F32R, kind="ExternalInput")
    wqk_d = nc.dram_tensor("wqk_t", [C, 512], F32R, kind="ExternalInput")
    bqk_d = nc.dram_tensor("bqk", [512], F32, kind="ExternalInput")
    wv_d = nc.dram_tensor("wv_t", [C, 256], F32R, kind="ExternalInput")
    bv_d = nc.dram_tensor("bv_bcast", [128, 256], F32, kind="ExternalInput")
    pj_d = nc.dram_tensor("projt", [64, 4, 512], F32R, kind="ExternalInput")
    y_d = nc.dram_tensor("y", [N, C], F32, kind="ExternalOutput")

    with tile.TileContext(nc) as tc:
        with ExitStack() as ctx:
            const = ctx.enter_context(tc.tile_pool(name="const", bufs=1))
            work = ctx.enter_context(tc.tile_pool(name="work", bufs=3))
            small = ctx.enter_context(tc.tile_pool(name="small", bufs=2))
            psum = ctx.enter_context(tc.tile_pool(name="psum", bufs=2, space="PSUM"))

            # ---- resident tensors ----
            xt_sb = const.tile([128, 4, N], F32R)        # [c_p, ct, n]
            wqk_sb = const.tile([128, 4, 512], F32R)     # [c_p, ct, o]  o: 0-255 q, 256-511 k
            bqk_sb = const.tile([128, 4], F32)           # [o_p, ot]
            wv_sb = const.tile([128, 4, 256], F32R)      # [c_p, ct, dv]
            bv_sb = const.tile([128, 256], F32)
            pj_sb = const.tile([64, 4, 512], F32R)       # [d, h, co]
            ones_sb = const.tile([128, 64], F32R)
            qk_sb = const.tile([128, 4, N], F32R)        # [o_p, ot, n] ot 0-1 q, 2-3 k
            v_sb = const.tile([128, MT, 4, 65], F32R)    # [m_p, mt, h, dv+1]
            outn_sb = const.tile([64, 4, N], F32R)       # [d, h, n]

            nc.sync.dma_start(wqk_sb[:, :, :], wqk_d.ap().rearrange("(t p) o -> p t o", p=128))
            nc.sync.dma_start(bqk_sb[:, :], bqk_d.ap().rearrange("(t p) -> p t", p=128))
            nc.sync.dma_start(wv_sb[:, :, :], wv_d.ap().rearrange("(t p) o -> p t o", p=128))
            nc.sync.dma_start(bv_sb[:, :], bv_d.ap())
            nc.sync.dma_start(pj_sb[:, :, :], pj_d.ap())
            for kt in range(4):
                nc.sync.dma_start(xt_sb[:, kt, :], xt_d.ap()[128 * kt : 128 * (kt + 1), :])
            ONE_F32_BITS = 0x3F800000
            nc.vector.memset(ones_sb[:, :].bitcast(mybir.dt.uint32), ONE_F32_BITS)
            nc.vector.memset(v_sb[:, :, :, 64:65].bitcast(mybir.dt.uint32), ONE_F32_BITS)

            # ---- phase 1a: q,k projection  qk^T[o, n] ----
            for ot in range(4):
                for nt in range(NT512):
                    ps = psum.tile([128, 2, 512], F32, tag="spair")
                    for kt in range(4):
                        nc.tensor.matmul(
                            ps[:, 0, :],
                            lhsT=wqk_sb[:, kt, 128 * ot : 128 * (ot + 1)],
                            rhs=xt_sb[:, kt, 512 * nt : 512 * (nt + 1)],
                            start=(kt == 0),
                            stop=(kt == 3),
                        )
                    nc.vector.tensor_scalar_add(
                        qk_sb[:, ot, 512 * nt : 512 * (nt + 1)],
                        ps[:, 0, :],
                        bqk_sb[:, ot : ot + 1],
                    )

            # ---- phase 1b: v projection  v[m, dv] (+bias), interleaved by head ----
            for mt in range(MT):
                ps = psum.tile([128, 2, 512], F32, tag="spair")
                for kt in range(4):
                    nc.tensor.matmul(
                        ps[:, 0, :256],
                        lhsT=xt_sb[:, kt, 128 * mt : 128 * (mt + 1)],
                        rhs=wv_sb[:, kt, :],
                        start=(kt == 0),
                        stop=(kt == 3),
                    )
                nc.vector.tensor_tensor(
                    out=v_sb[:, mt, :, 0:64],
                    in0=ps[:, 0, :256].rearrange("p (h d) -> p h d", h=4),
                    in1=bv_sb[:, :].rearrange("p (h d) -> p h d", h=4),
                    op=mybir.AluOpType.add,
                )

            # ---- phase 2: attention, head pairs (0,1) and (2,3) ----
            for nci in range(NT512):
                nsl = slice(512 * nci, 512 * (nci + 1))
                for pair in range(2):
                    heads = (2 * pair, 2 * pair + 1)
                    ot_ps = psum.tile([65, 2, 512], F32, tag="outT")
                    for mt in range(MT):
                        sp = psum.tile([128, 2, 512], F32, tag="spair")
                        for hi, h in enumerate(heads):
                            par = h % 2
                            otq = h // 2
                            psl = slice(64 * par, 64 * par + 64)
                            nc.tensor.matmul(
                                sp[:, hi, :],
                                lhsT=qk_sb[psl, 2 + otq, 128 * mt : 128 * (mt + 1)],
                                rhs=qk_sb[psl, otq, nsl],
                                start=True,
                                stop=True,
                            )
                        pt = work.tile([128, 2, 512], F32R, tag="ptile")
                        nc.scalar.activation(
                            pt[:, :, :],
                            sp[:, :, :],
                            mybir.ActivationFunctionType.Exp,
                            scale=float(EXP_SCALE),
                        )
                        for hi, h in enumerate(heads):
                            nc.tensor.matmul(
                                ot_ps[:, hi, :],
                                lhsT=v_sb[:, mt, h, :],
                                rhs=pt[:, hi, :],
                                start=(mt == 0),
                                stop=(mt == MT - 1),
                            )
                    # normalization for this (pair, n-chunk)
                    for hi, h in enumerate(heads):
                        rc = small.tile([128, 512], F32R, tag="recip")
                        with nc.allow_low_precision(reason="f32r holds full fp32 bits"):
                            nc.vector.reciprocal(rc[64:65, :], ot_ps[64:65, hi, :])
                        bc = psum.tile([128, 2, 512], F32, tag="spair")
                        nc.tensor.matmul(
                            bc[0:64, 0, :],
                            lhsT=ones_sb[64:65, 0:64],
                            rhs=rc[64:65, :],
                            start=True,
                            stop=True,
                        )
                        bc_sb = small.tile([64, 512], F32, tag="bcsb")
                        nc.vector.tensor_copy(bc_sb[:, :], bc[0:64, 0, :])
                        nc.vector.tensor_tensor(
                            out=outn_sb[0:64, h, nsl],
                            in0=ot_ps[0:64, hi, :],
                            in1=bc_sb[:, :],
                            op=mybir.AluOpType.mult,
                        )
                # ---- phase 3: projection partial for the 4 n-tiles of this chunk ----
                for ntl in range(4):
                    nt = 4 * nci + ntl
                    yp = psum.tile([128, 2, 512], F32, tag="spair")
                    for h in range(H_PER_CORE):
                        nc.tensor.matmul(
                            yp[:, 0, :],
                            lhsT=outn_sb[0:64, h, 128 * nt : 128 * (nt + 1)],
                            rhs=pj_sb[0:64, h, :],
                            start=(h == 0),
                            stop=(h == 3),
                        )
                    yt = small.tile([128, 512], F32, tag="ytile")
                    nc.vector.tensor_copy(yt[:, :], yp[:, 0, :])
                    nc.sync.dma_start(y_d.ap()[128 * nt : 128 * (nt + 1), :], yt[:, :])

    nc.compile()
    return nc


_NC = None


def _get_nc():
    global _NC
    if _NC is None:
        _NC = build_nc()
    return _NC


def make_in_maps(x, qkv_w, qkv_b, proj_w):
    x = np.asarray(x, dtype=np.float32)
    qkv_w = np.asarray(qkv_w, dtype=np.float32)
    qkv_b = np.asarray(qkv_b, dtype=np.float32)
    proj_w = np.asarray(proj_w, dtype=np.float32)
    in_maps = []
    for core in range(8):
        b, g = divmod(core, 2)
        o0 = 256 * g
        wqk = np.concatenate([qkv_w[o0 : o0 + 256], qkv_w[512 + o0 : 512 + o0 + 256]], axis=0)
        bqk = np.concatenate([qkv_b[o0 : o0 + 256], qkv_b[512 + o0 : 512 + o0 + 256]], axis=0)
        wv = qkv_w[1024 + o0 : 1024 + o0 + 256]
        bv = qkv_b[1024 + o0 : 1024 + o0 + 256]
        pj = proj_w[:, o0 : o0 + 256]  # [512(co), 256(ci)]
        projt = np.ascontiguousarray(
            pj.T.reshape(4, 64, 512).transpose(1, 0, 2)
        )  # [64(d), 4(h), 512(co)]
        in_maps.append(
            {
                "xt": np.ascontiguousarray(x[b].T),
                "wqk_t": np.ascontiguousarray(wqk.T),
                "bqk": np.ascontiguousarray(bqk),
                "wv_t": np.ascontiguousarray(wv.T),
                "bv_bcast": np.ascontiguousarray(np.tile(bv[None, :], (128, 1))),
                "projt": projt,
            }
        )
    return in_maps


def kernel(x, qkv_w, qkv_b, proj_w, proj_b, _trace=False, _trace_kwargs=None):
    nc = _get_nc()
    in_maps = make_in_maps(x, qkv_w, qkv_b, proj_w)
    kw = {}
    if _trace:
        kw = {"trace": True, **(_trace_kwargs or {})}
    res = run_bass_kernel_spmd(nc, in_maps, core_ids=list(range(8)), **kw)
    proj_b = np.asarray(proj_b, dtype=np.float32)
    y = np.empty((B, N, C), dtype=np.float32)
    for b in range(B):
        y[b] = res.results[2 * b]["y"] + res.results[2 * b + 1]["y"] + proj_b
    kernel.last_results = res
    return y


# revision 5
# speedup vs baseline: 1.1706x; 1.1706x over previous
"""Multi-head attention forward (B=4, N=2048, C=512, H=8, D=64) on 8 TRN2 cores.

Sharding: core = 2*b + g  (b = batch 0..3, g = head-group 0..1, 4 heads each).
Each core computes a partial projection output for its batch from its 4 heads;
the host sums the two group partials and adds proj bias.

Per-core dataflow (all layouts chosen so no on-device transposes are needed):
  xT [C, N] resident in SBUF (bf16).
  qk^T = Wqk^T.T @ x^T         -> [512(o), 2048(n)]  (o on partitions)
  v    = x^T.T @ Wv^T          -> [2048(m), 256(dv)] (m on partitions) + ones col
  per head pair (parity packs the 64-wide contraction into both PE row groups):
    S^T chunk [128(m), 2, 512(n)] = k^T.T @ q^T   (K=64 row-tiled matmuls)
    P^T = exp(S^T * 0.125)  on ACT engine, PSUM -> SBUF bf16, one instr per pair
    outT'[65, n] += V_aug.T @ P^T   (row 64 accumulates softmax denominator)
  normalize: copy sm row to SBUF, K=1 matmul broadcasts it across 64
  partitions, 64-lane reciprocal, multiply -> outT_norm [64(d), 4(h), 2048(n)].
  y_partial[n, co] = sum_h outT_norm_h.T @ projT_h  (K=64 accumulated in PSUM)

Matmul operands are bf16 (1 row/cycle + fast weight load); accumulation is
fp32 in PSUM. The K=1 broadcast matmul runs in float32r to keep the softmax
denominator exact.
"""

from contextlib import ExitStack

import ml_dtypes
import numpy as np

import concourse.bass as bass
import concourse.mybir as mybir
import concourse.tile as tile
from concourse import bacc
from concourse.bass_utils import run_bass_kernel_spmd

F32 = mybir.dt.float32
F32R = mybir.dt.float32r
BF16 = mybir.dt.bfloat16

B, N, C = 4, 2048, 512
H_PER_CORE = 4
D = 64
NT512 = N // 512          # 4 chunks of 512 along n
MT = N // 128             # 16 m tiles
EXP_SCALE = 1.0 / np.sqrt(D)


def build_nc():
    nc = bacc.Bacc(
        "TRN2",
        target_bir_lowering=False,
        debug=False,
        enable_asserts=False,
        num_devices=8,
    )

    xt_d = nc.dram_tensor("xt", [C, N], BF16, kind="ExternalInput")
    wqk_d = nc.dram_tensor("wqk_t", [C, 512], BF16, kind="ExternalInput")
    bqk_d = nc.dram_tensor("bqk", [512], F32, kind="ExternalInput")
    wv_d = nc.dram_tensor("wv_t", [C, 256], BF16, kind="ExternalInput")
    bv_d = nc.dram_tensor("bv_bcast", [128, 256], F32, kind="ExternalInput")
    pj_d = nc.dram_tensor("projt", [64, 4, 512], BF16, kind="ExternalInput")
    y_d = nc.dram_tensor("y", [N, C], F32, kind="ExternalOutput")

    with tile.TileContext(nc) as tc:
        with ExitStack() as ctx:
            const = ctx.enter_context(tc.tile_pool(name="const", bufs=1))
            work = ctx.enter_context(tc.tile_pool(name="work", bufs=3))
            small = ctx.enter_context(tc.tile_pool(name="small", bufs=2))
            psum = ctx.enter_context(tc.tile_pool(name="psum", bufs=2, space="PSUM"))

            # ---- resident tensors ----
            xt_sb = const.tile([128, 4, N], BF16)        # [c_p, ct, n]
            wqk_sb = const.tile([128, 4, 512], BF16)     # [c_p, ct, o]  o: 0-255 q, 256-511 k
            bqk_sb = const.tile([128, 4], F32)           # [o_p, ot]
            wv_sb = const.tile([128, 4, 256], BF16)      # [c_p, ct, dv]
            bv_sb = const.tile([128, 256], F32)
            pj_sb = const.tile([64, 4, 512], BF16)       # [d, h, co]
            ones_sb = const.tile([128, 64], F32R)
            qk_sb = const.tile([128, 4, N], BF16)        # [o_p, ot, n] ot 0-1 q, 2-3 k
            v_sb = const.tile([128, MT, 4, 65], BF16)    # [m_p, mt, h, dv+1]
            outn_sb = const.tile([64, 4, N], BF16)       # [d, h, n]

            nc.sync.dma_start(wqk_sb[:, :, :], wqk_d.ap().rearrange("(t p) o -> p t o", p=128))
            nc.sync.dma_start(bqk_sb[:, :], bqk_d.ap().rearrange("(t p) -> p t", p=128))
            nc.sync.dma_start(wv_sb[:, :, :], wv_d.ap().rearrange("(t p) o -> p t o", p=128))
            nc.sync.dma_start(bv_sb[:, :], bv_d.ap())
            nc.sync.dma_start(pj_sb[:, :, :], pj_d.ap())
            for kt in range(4):
                nc.sync.dma_start(xt_sb[:, kt, :], xt_d.ap()[128 * kt : 128 * (kt + 1), :])
            ONE_F32_BITS = 0x3F800000
            nc.vector.memset(ones_sb[:, :].bitcast(mybir.dt.uint32), ONE_F32_BITS)
            nc.vector.memset(v_sb[:, :, :, 64:65], 1.0)

            # ---- phase 1a: q,k projection  qk^T[o, n] ----
            for ot in range(4):
                for nt in range(NT512):
                    ps = psum.tile([128, 2, 512], F32, tag="spair")
                    for kt in range(4):
                        nc.tensor.matmul(
                            ps[:, 0, :],
                            lhsT=wqk_sb[:, kt, 128 * ot : 128 * (ot + 1)],
                            rhs=xt_sb[:, kt, 512 * nt : 512 * (nt + 1)],
                            start=(kt == 0),
                            stop=(kt == 3),
                        )
                    nc.vector.tensor_scalar_add(
                        qk_sb[:, ot, 512 * nt : 512 * (nt + 1)],
                        ps[:, 0, :],
                        bqk_sb[:, ot : ot + 1],
                    )

            # ---- phase 1b: v projection  v[m, dv] (+bias), ones col at dv=64 ----
            for mt in range(MT):
                ps = psum.tile([128, 2, 512], F32, tag="spair")
                for kt in range(4):
                    nc.tensor.matmul(
                        ps[:, 0, :256],
                        lhsT=xt_sb[:, kt, 128 * mt : 128 * (mt + 1)],
                        rhs=wv_sb[:, kt, :],
                        start=(kt == 0),
                        stop=(kt == 3),
                    )
                nc.vector.tensor_tensor(
                    out=v_sb[:, mt, :, 0:64],
                    in0=ps[:, 0, :256].rearrange("p (h d) -> p h d", h=4),
                    in1=bv_sb[:, :].rearrange("p (h d) -> p h d", h=4),
                    op=mybir.AluOpType.add,
                )

            # ---- phase 2: attention, head pairs (0,1) and (2,3) ----
            for nci in range(NT512):
                nsl = slice(512 * nci, 512 * (nci + 1))
                for pair in range(2):
                    heads = (2 * pair, 2 * pair + 1)
                    ot_ps = psum.tile([65, 2, 512], F32, tag="outT")
                    for mt in range(MT):
                        sp = psum.tile([128, 2, 512], F32, tag="spair")
                        for hi, h in enumerate(heads):
                            par = h % 2
                            otq = h // 2
                            psl = slice(64 * par, 64 * par + 64)
                            nc.tensor.matmul(
                                sp[:, hi, :],
                                lhsT=qk_sb[psl, 2 + otq, 128 * mt : 128 * (mt + 1)],
                                rhs=qk_sb[psl, otq, nsl],
                                start=True,
                                stop=True,
                            )
                        pt = work.tile([128, 2, 512], BF16, tag="ptile")
                        nc.scalar.activation(
                            pt[:, :, :],
                            sp[:, :, :],
                            mybir.ActivationFunctionType.Exp,
                            scale=float(EXP_SCALE),
                        )
                        for hi, h in enumerate(heads):
                            nc.tensor.matmul(
                                ot_ps[:, hi, :],
                                lhsT=v_sb[:, mt, h, :],
                                rhs=pt[:, hi, :],
                                start=(mt == 0),
                                stop=(mt == MT - 1),
                            )
                    # normalization for this (pair, n-chunk)
                    for hi, h in enumerate(heads):
                        sm_sb = small.tile([128, 512], F32R, tag="smsb")
                        nc.vector.tensor_copy(sm_sb[64:65, :], ot_ps[64:65, hi, :])
                        bc = psum.tile([128, 2, 512], F32, tag="spair")
                        nc.tensor.matmul(
                            bc[0:64, 0, :],
                            lhsT=ones_sb[64:65, 0:64],
                            rhs=sm_sb[64:65, :],
                            start=True,
                            stop=True,
                        )
                        bc_sb = small.tile([64, 512], F32, tag="bcsb")
                        nc.vector.reciprocal(bc_sb[:, :], bc[0:64, 0, :])
                        nc.vector.tensor_tensor(
                            out=outn_sb[0:64, h, nsl],
                            in0=ot_ps[0:64, hi, :],
                            in1=bc_sb[:, :],
                            op=mybir.AluOpType.mult,
                        )
                # ---- phase 3: projection partial for the 4 n-tiles of this chunk ----
                for ntl in range(4):
                    nt = 4 * nci + ntl
                    yp = psum.tile([128, 2, 512], F32, tag="spair")
                    for h in range(H_PER_CORE):
                        nc.tensor.matmul(
                            yp[:, 0, :],
                            lhsT=outn_sb[0:64, h, 128 * nt : 128 * (nt + 1)],
                            rhs=pj_sb[0:64, h, :],
                            start=(h == 0),
                            stop=(h == 3),
                        )
                    yt = small.tile([128, 512], F32, tag="ytile")
                    nc.vector.tensor_copy(yt[:, :], yp[:, 0, :])
                    nc.sync.dma_start(y_d.ap()[128 * nt : 128 * (nt + 1), :], yt[:, :])

    nc.compile()
    return nc


_NC = None


def _get_nc():
    global _NC
    if _NC is None:
        _NC = build_nc()
    return _NC


def make_in_maps(x, qkv_w, qkv_b, proj_w):
    x = np.asarray(x, dtype=np.float32)
    qkv_w = np.asarray(qkv_w, dtype=np.float32)
    qkv_b = np.asarray(qkv_b, dtype=np.float32)
    proj_w = np.asarray(proj_w, dtype=np.float32)
    bf16 = ml_dtypes.bfloat16
    in_maps = []
    for core in range(8):
        b, g = divmod(core, 2)
        o0 = 256 * g
        wqk = np.concatenate([qkv_w[o0 : o0 + 256], qkv_w[512 + o0 : 512 + o0 + 256]], axis=0)
        bqk = np.concatenate([qkv_b[o0 : o0 + 256], qkv_b[512 + o0 : 512 + o0 + 256]], axis=0)
        wv = qkv_w[1024 + o0 : 1024 + o0 + 256]
        bv = qkv_b[1024 + o0 : 1024 + o0 + 256]
        pj = proj_w[:, o0 : o0 + 256]  # [512(co), 256(ci)]
        projt = np.ascontiguousarray(
            pj.T.reshape(4, 64, 512).transpose(1, 0, 2)
        ).astype(bf16)  # [64(d), 4(h), 512(co)]
        in_maps.append(
            {
                "xt": np.ascontiguousarray(x[b].T).astype(bf16),
                "wqk_t": np.ascontiguousarray(wqk.T).astype(bf16),
                "bqk": np.ascontiguousarray(bqk),
                "wv_t": np.ascontiguousarray(wv.T).astype(bf16),
                "bv_bcast": np.ascontiguousarray(np.tile(bv[None, :], (128, 1))),
                "projt": projt,
            }
        )
    return in_maps


def kernel(x, qkv_w, qkv_b, proj_w, proj_b, _trace=False, _trace_kwargs=None):
    nc = _get_nc()
    in_maps = make_in_maps(x, qkv_w, qkv_b, proj_w)
    kw = {}
    if _trace:
        kw = {"trace": True, **(_trace_kwargs or {})}
    res = run_bass_kernel_spmd(nc, in_maps, core_ids=list(range(8)), **kw)
    proj_b = np.asarray(proj_b, dtype=np.float32)
    y = np.empty((B, N, C), dtype=np.float32)
    for b in range(B):
        y[b] = res.results[2 * b]["y"] + res.results[2 * b + 1]["y"] + proj_b
    kernel.last_results = res
    return y


# revision 7
# speedup vs baseline: 1.2387x; 1.0582x over previous
"""Multi-head attention forward (B=4, N=2048, C=512, H=8, D=64) on 8 TRN2 cores.

Sharding: core = 2*b + g  (b = batch 0..3, g = head-group 0..1, 4 heads each).
Each core computes a partial projection output for its batch from its 4 heads;
the host sums the two group partials and adds proj bias.

Per-core dataflow (all layouts chosen so no on-device transposes are needed):
  xT [C, N] resident in SBUF (bf16).
  qk^T = Wqk^T.T @ x^T         -> [512(o), 2048(n)]  (o on partitions)
  v    = x^T.T @ Wv^T          -> [2048(m), 256(dv)] (m on partitions) + ones col
  per head pair (parity packs the 64-wide contraction into both PE row groups):
    S^T chunk [128(m), 2, 512(n)] = k^T.T @ q^T   (K=64 row-tiled matmuls)
    P^T = exp(S^T * 0.125)  on ACT engine, PSUM -> SBUF bf16, one instr per pair
    outT'[65, n] += V_aug.T @ P^T   (row 64 accumulates softmax denominator)
  normalize: copy sm row to SBUF, K=1 matmul broadcasts it across 64
  partitions, 64-lane reciprocal, multiply -> outT_norm [64(d), 4(h), 2048(n)].
  y_partial[n, co] = sum_h outT_norm_h.T @ projT_h  (K=64 accumulated in PSUM)

Matmul operands are bf16 (1 row/cycle + fast weight load); accumulation is
fp32 in PSUM. The K=1 broadcast matmul runs in float32r to keep the softmax
denominator exact.
"""

from contextlib import ExitStack

import ml_dtypes
import numpy as np

import concourse.bass as bass
import concourse.mybir as mybir
import concourse.tile as tile
from concourse import bacc
from concourse.bass_utils import run_bass_kernel_spmd

F32 = mybir.dt.float32
F32R = mybir.dt.float32r
BF16 = mybir.dt.bfloat16

B, N, C = 4, 2048, 512
H_PER_CORE = 4
D = 64
NT512 = N // 512          # 4 chunks of 512 along n
MT = N // 128             # 16 m tiles
EXP_SCALE = 1.0 / np.sqrt(D)


def build_nc():
    nc = bacc.Bacc(
        "TRN2",
        target_bir_lowering=False,
        debug=False,
        enable_asserts=False,
        num_devices=8,
    )

    xt_d = nc.dram_tensor("xt", [C, N], BF16, kind="ExternalInput")
    wqk_d = nc.dram_tensor("wqk_t", [C, 512], BF16, kind="ExternalInput")
    bqk_d = nc.dram_tensor("bqk", [512], F32, kind="ExternalInput")
    wv_d = nc.dram_tensor("wv_t", [C, 256], BF16, kind="ExternalInput")
    bv_d = nc.dram_tensor("bv_bcast", [128, 256], F32, kind="ExternalInput")
    pj_d = nc.dram_tensor("projt", [64, 4, 512], BF16, kind="ExternalInput")
    y_d = nc.dram_tensor("y", [N, C], F32, kind="ExternalOutput")

    with tile.TileContext(nc) as tc:
        with ExitStack() as ctx:
            const = ctx.enter_context(tc.tile_pool(name="const", bufs=1))
            work = ctx.enter_context(tc.tile_pool(name="work", bufs=3))
            small = ctx.enter_context(tc.tile_pool(name="small", bufs=2))
            psum = ctx.enter_context(tc.tile_pool(name="psum", bufs=2, space="PSUM"))

            # ---- resident tensors ----
            xt_sb = const.tile([128, 4, N], BF16)        # [c_p, ct, n]
            wqk_sb = const.tile([128, 4, 512], BF16)     # [c_p, ct, o]  o: 0-255 q, 256-511 k
            bqk_sb = const.tile([128, 4], F32)           # [o_p, ot]
            wv_sb = const.tile([128, 4, 256], BF16)      # [c_p, ct, dv]
            bv_sb = const.tile([128, 256], F32)
            pj_sb = const.tile([64, 4, 512], BF16)       # [d, h, co]
            ones_sb = const.tile([128, 64], F32R)
            qk_sb = const.tile([128, 4, N], BF16)        # [o_p, ot, n] ot 0-1 q, 2-3 k
            v_sb = const.tile([128, MT, 4, 65], BF16)    # [m_p, mt, h, dv+1]
            outn_sb = const.tile([64, 4, N], BF16)       # [d, h, n]

            nc.sync.dma_start(wqk_sb[:, :, :], wqk_d.ap().rearrange("(t p) o -> p t o", p=128))
            nc.sync.dma_start(bqk_sb[:, :], bqk_d.ap().rearrange("(t p) -> p t", p=128))
            nc.sync.dma_start(wv_sb[:, :, :], wv_d.ap().rearrange("(t p) o -> p t o", p=128))
            nc.sync.dma_start(bv_sb[:, :], bv_d.ap())
            nc.sync.dma_start(pj_sb[:, :, :], pj_d.ap())
            for nt in range(NT512):
                for kt in range(4):
                    nc.sync.dma_start(
                        xt_sb[:, kt, 512 * nt : 512 * (nt + 1)],
                        xt_d.ap()[128 * kt : 128 * (kt + 1), 512 * nt : 512 * (nt + 1)],
                    )
            ONE_F32_BITS = 0x3F800000
            nc.vector.memset(ones_sb[:, :].bitcast(mybir.dt.uint32), ONE_F32_BITS)
            nc.vector.memset(v_sb[:, :, :, 64:65], 1.0)

            # ---- phase 1a: q,k projection  qk^T[o, n] ----
            for ot in range(4):
                for nt in range(NT512):
                    ps = psum.tile([128, 2, 512], F32, tag="spair")
                    for kt in range(4):
                        nc.tensor.matmul(
                            ps[:, 0, :],
                            lhsT=wqk_sb[:, kt, 128 * ot : 128 * (ot + 1)],
                            rhs=xt_sb[:, kt, 512 * nt : 512 * (nt + 1)],
                            start=(kt == 0),
                            stop=(kt == 3),
                        )
                    nc.vector.tensor_scalar_add(
                        qk_sb[:, ot, 512 * nt : 512 * (nt + 1)],
                        ps[:, 0, :],
                        bqk_sb[:, ot : ot + 1],
                    )

            # ---- phase 1b: v projection  v[m, dv] (+bias), ones col at dv=64 ----
            for mt in range(MT):
                ps = psum.tile([128, 2, 512], F32, tag="spair")
                for kt in range(4):
                    nc.tensor.matmul(
                        ps[:, 0, :256],
                        lhsT=xt_sb[:, kt, 128 * mt : 128 * (mt + 1)],
                        rhs=wv_sb[:, kt, :],
                        start=(kt == 0),
                        stop=(kt == 3),
                    )
                nc.vector.tensor_tensor(
                    out=v_sb[:, mt, :, 0:64],
                    in0=ps[:, 0, :256].rearrange("p (h d) -> p h d", h=4),
                    in1=bv_sb[:, :].rearrange("p (h d) -> p h d", h=4),
                    op=mybir.AluOpType.add,
                )

            # ---- phases 2+3: attention with software-pipelined normalize ----
            # The normalize chain (DVE-bound) and the y projection for block i
            # are emitted in the middle of block i+1's mt loop so the PE queue
            # (strict FIFO) never drains behind the DVE at block boundaries.
            def emit_normalize(st):
                nci, pair, heads, ot_ps, nsl = st
                for hi, h in enumerate(heads):
                    sm_sb = small.tile([128, 512], F32R, tag="smsb")
                    nc.vector.tensor_copy(sm_sb[64:65, :], ot_ps[64:65, hi, :])
                    bc = psum.tile([128, 2, 512], F32, tag="spair")
                    nc.tensor.matmul(
                        bc[0:64, 0, :],
                        lhsT=ones_sb[64:65, 0:64],
                        rhs=sm_sb[64:65, :],
                        start=True,
                        stop=True,
                    )
                    bc_sb = small.tile([64, 512], F32, tag="bcsb")
                    nc.vector.reciprocal(bc_sb[:, :], bc[0:64, 0, :])
                    nc.vector.tensor_tensor(
                        out=outn_sb[0:64, h, nsl],
                        in0=ot_ps[0:64, hi, :],
                        in1=bc_sb[:, :],
                        op=mybir.AluOpType.mult,
                    )
                if pair == 1:
                    for ntl in range(4):
                        nt = 4 * nci + ntl
                        yp = psum.tile([128, 2, 512], F32, tag="spair")
                        for h in range(H_PER_CORE):
                            nc.tensor.matmul(
                                yp[:, 0, :],
                                lhsT=outn_sb[0:64, h, 128 * nt : 128 * (nt + 1)],
                                rhs=pj_sb[0:64, h, :],
                                start=(h == 0),
                                stop=(h == 3),
                            )
                        yt = small.tile([128, 512], F32, tag="ytile")
                        nc.vector.tensor_copy(yt[:, :], yp[:, 0, :])
                        nc.sync.dma_start(y_d.ap()[128 * nt : 128 * (nt + 1), :], yt[:, :])

            pending = None
            for nci in range(NT512):
                nsl = slice(512 * nci, 512 * (nci + 1))
                for pair in range(2):
                    heads = (2 * pair, 2 * pair + 1)
                    ot_ps = psum.tile([65, 2, 512], F32, tag="outT")
                    for mt in range(MT):
                        sp = psum.tile([128, 2, 512], F32, tag="spair")
                        for hi, h in enumerate(heads):
                            par = h % 2
                            otq = h // 2
                            psl = slice(64 * par, 64 * par + 64)
                            nc.tensor.matmul(
                                sp[:, hi, :],
                                lhsT=qk_sb[psl, 2 + otq, 128 * mt : 128 * (mt + 1)],
                                rhs=qk_sb[psl, otq, nsl],
                                start=True,
                                stop=True,
                            )
                        pt = work.tile([128, 2, 512], BF16, tag="ptile")
                        nc.scalar.activation(
                            pt[:, :, :],
                            sp[:, :, :],
                            mybir.ActivationFunctionType.Exp,
                            scale=float(EXP_SCALE),
                        )
                        for hi, h in enumerate(heads):
                            nc.tensor.matmul(
                                ot_ps[:, hi, :],
                                lhsT=v_sb[:, mt, h, :],
                                rhs=pt[:, hi, :],
                                start=(mt == 0),
                                stop=(mt == MT - 1),
                            )
                        if mt == 3 and pending is not None:
                            emit_normalize(pending)
                            pending = None
                    pending = (nci, pair, heads, ot_ps, nsl)
            emit_normalize(pending)

    nc.compile()
    return nc


_NC = None


def _get_nc():
    global _NC
    if _NC is None:
        _NC = build_nc()
    return _NC


def make_in_maps(x, qkv_w, qkv_b, proj_w):
    x = np.asarray(x, dtype=np.float32)
    qkv_w = np.asarray(qkv_w, dtype=np.float32)
    qkv_b = np.asarray(qkv_b, dtype=np.float32)
    proj_w = np.asarray(proj_w, dtype=np.float32)
    bf16 = ml_dtypes.bfloat16
    in_maps = []
    for core in range(8):
        b, g = divmod(core, 2)
        o0 = 256 * g
        wqk = np.concatenate([qkv_w[o0 : o0 + 256], qkv_w[512 + o0 : 512 + o0 + 256]], axis=0)
        bqk = np.concatenate([qkv_b[o0 : o0 + 256], qkv_b[512 + o0 : 512 + o0 + 256]], axis=0)
        wv = qkv_w[1024 + o0 : 1024 + o0 + 256]
        bv = qkv_b[1024 + o0 : 1024 + o0 + 256]
        pj = proj_w[:, o0 : o0 + 256]  # [512(co), 256(ci)]
        projt = np.ascontiguousarray(
            pj.T.reshape(4, 64, 512).transpose(1, 0, 2)
        ).astype(bf16)  # [64(d), 4(h), 512(co)]
        in_maps.append(
            {
                "xt": np.ascontiguousarray(x[b].T).astype(bf16),
                "wqk_t": np.ascontiguousarray(wqk.T).astype(bf16),
                "bqk": np.ascontiguousarray(bqk),
                "wv_t": np.ascontiguousarray(wv.T).astype(bf16),
                "bv_bcast": np.ascontiguousarray(np.tile(bv[None, :], (128, 1))),
                "projt": projt,
            }
        )
    return in_maps


def kernel(x, qkv_w, qkv_b, proj_w, proj_b, _trace=False, _trace_kwargs=None):
    nc = _get_nc()
    in_maps = make_in_maps(x, qkv_w, qkv_b, proj_w)
    kw = {}
    if _trace:
        kw = {"trace": True, **(_trace_kwargs or {})}
    res = run_bass_kernel_spmd(nc, in_maps, core_ids=list(range(8)), **kw)
    proj_b = np.asarray(proj_b, dtype=np.float32)
    y = np.empty((B, N, C), dtype=np.float32)
    for b in range(B):
        y[b] = res.results[2 * b]["y"] + res.results[2 * b + 1]["y"] + proj_b
    kernel.last_results = res
    return y


# revision 9
# speedup vs baseline: 1.3652x; 1.1021x over previous
"""Multi-head attention forward (B=4, N=2048, C=512, H=8, D=64) on 8 TRN2 cores.

Sharding: core = 2*b + g  (b = batch 0..3, g = head-group 0..1, 4 heads each).
Each core computes a partial projection output for its batch from its 4 heads;
the host sums the two group partials and adds proj bias.

Per-core dataflow (layouts chosen so no on-device transposes are needed):
  xT [C, N] resident in SBUF (bf16).
  qk^T = Wqk^T.T @ x^T         -> [512(o), 2048(n)]  (o on partitions)
  v    = x^T.T @ Wv^T          -> [2048(m), 256(dv)] (m on partitions) + ones col
  per head pair (parity packs the 64-wide contraction into both PE row groups):
    S^T chunk [128(m), 2, 512(n)] = k^T.T @ q^T   (K=64 row-tiled matmuls)
    P^T = exp(S^T * 0.125)  on ACT engine, PSUM -> SBUF bf16, one instr per pair
    outT'[65, n] += V_aug.T @ P^T   (row 64 accumulates softmax denominator)
  normalize: reciprocal of PSUM row 64, K=1 matmul broadcasts it across 64
  partitions, multiply -> outT_norm [64(d), 4(h), 2048(n)].
  y_partial[n, co] = sum_h outT_norm_h.T @ projT_h  (K=64 accumulated in PSUM)

The kernel is paced by the ACT engine (exp is irreducible: N^2 elements at 1
elem/lane/cycle), so everything else is software-pipelined around it:
 - block i's normalize + projection work is spread piecewise across block
   i+1's mt loop so the PE FIFO never stalls behind the DVE reciprocal;
 - the v projection and half the qkv projection are emitted just-in-time
   inside block 0 so the first exp starts as early as possible.
Matmul operands are bf16 (1 row/cycle + fast weight load); accumulation is
fp32 in PSUM; the K=1 broadcast matmul runs in float32r on full-fp32 bits.
"""

from contextlib import ExitStack

import ml_dtypes
import numpy as np

import concourse.bass as bass
import concourse.mybir as mybir
import concourse.tile as tile
from concourse import bacc
from concourse.bass_utils import run_bass_kernel_spmd

F32 = mybir.dt.float32
F32R = mybir.dt.float32r
BF16 = mybir.dt.bfloat16

B, N, C = 4, 2048, 512
H_PER_CORE = 4
D = 64
NT512 = N // 512          # 4 chunks of 512 along n
MT = N // 128             # 16 m tiles
EXP_SCALE = 1.0 / np.sqrt(D)


def build_nc():
    nc = bacc.Bacc(
        "TRN2",
        target_bir_lowering=False,
        debug=False,
        enable_asserts=False,
        num_devices=8,
    )

    xt_d = nc.dram_tensor("xt", [C, N], BF16, kind="ExternalInput")
    wqk_d = nc.dram_tensor("wqk_t", [C, 512], BF16, kind="ExternalInput")
    bqk_d = nc.dram_tensor("bqk", [512], F32, kind="ExternalInput")
    wv_d = nc.dram_tensor("wv_t", [C, 256], BF16, kind="ExternalInput")
    bv_d = nc.dram_tensor("bv_bcast", [128, 256], F32, kind="ExternalInput")
    pj_d = nc.dram_tensor("projt", [64, 4, 512], BF16, kind="ExternalInput")
    y_d = nc.dram_tensor("y", [N, C], F32, kind="ExternalOutput")

    with tile.TileContext(nc) as tc:
        with ExitStack() as ctx:
            const = ctx.enter_context(tc.tile_pool(name="const", bufs=1))
            work = ctx.enter_context(tc.tile_pool(name="work", bufs=3))
            small = ctx.enter_context(tc.tile_pool(name="small", bufs=2))
            psum = ctx.enter_context(tc.tile_pool(name="psum", bufs=2, space="PSUM"))

            # ---- resident tensors ----
            xt_sb = const.tile([128, 4, N], BF16)        # [c_p, ct, n]
            wqk_sb = const.tile([128, 4, 512], BF16)     # [c_p, ct, o]  o: 0-255 q, 256-511 k
            bqk_sb = const.tile([128, 4], F32)           # [o_p, ot]
            wv_sb = const.tile([128, 4, 256], BF16)      # [c_p, ct, dv]
            bv_sb = const.tile([128, 256], F32)
            pj_sb = const.tile([64, 4, 512], BF16)       # [d, h, co]
            ones_sb = const.tile([128, 64], F32R)
            qk_sb = const.tile([128, 4, N], BF16)        # [o_p, ot, n] ot 0-1 q, 2-3 k
            v_sb = const.tile([128, MT, 4, 65], BF16)    # [m_p, mt, h, dv+1]
            outn_sb = const.tile([64, 4, N], BF16)       # [d, h, n]

            nc.sync.dma_start(wqk_sb[:, :, :], wqk_d.ap().rearrange("(t p) o -> p t o", p=128))
            nc.sync.dma_start(bqk_sb[:, :], bqk_d.ap().rearrange("(t p) -> p t", p=128))
            nc.sync.dma_start(wv_sb[:, :, :], wv_d.ap().rearrange("(t p) o -> p t o", p=128))
            nc.sync.dma_start(bv_sb[:, :], bv_d.ap())
            nc.sync.dma_start(pj_sb[:, :, :], pj_d.ap())
            for nt in range(NT512):
                for kt in range(4):
                    nc.sync.dma_start(
                        xt_sb[:, kt, 512 * nt : 512 * (nt + 1)],
                        xt_d.ap()[128 * kt : 128 * (kt + 1), 512 * nt : 512 * (nt + 1)],
                    )
            ONE_F32_BITS = 0x3F800000
            nc.vector.memset(ones_sb[:, :].bitcast(mybir.dt.uint32), ONE_F32_BITS)
            nc.vector.memset(v_sb[:, :, :, 64:65], 1.0)

            def emit_qkv_chunk(ot, nt):
                ps = psum.tile([128, 2, 512], F32, tag="spair")
                for kt in range(4):
                    nc.tensor.matmul(
                        ps[:, 0, :],
                        lhsT=wqk_sb[:, kt, 128 * ot : 128 * (ot + 1)],
                        rhs=xt_sb[:, kt, 512 * nt : 512 * (nt + 1)],
                        start=(kt == 0),
                        stop=(kt == 3),
                    )
                nc.vector.tensor_scalar_add(
                    qk_sb[:, ot, 512 * nt : 512 * (nt + 1)],
                    ps[:, 0, :],
                    bqk_sb[:, ot : ot + 1],
                )

            def emit_v_chunk(mt):
                ps = psum.tile([128, 2, 512], F32, tag="spair")
                for kt in range(4):
                    nc.tensor.matmul(
                        ps[:, 0, :256],
                        lhsT=xt_sb[:, kt, 128 * mt : 128 * (mt + 1)],
                        rhs=wv_sb[:, kt, :],
                        start=(kt == 0),
                        stop=(kt == 3),
                    )
                nc.vector.tensor_tensor(
                    out=v_sb[:, mt, :, 0:64],
                    in0=ps[:, 0, :256].rearrange("p (h d) -> p h d", h=4),
                    in1=bv_sb[:, :].rearrange("p (h d) -> p h d", h=4),
                    op=mybir.AluOpType.add,
                )

            def emit_y(nt):
                yp = psum.tile([128, 2, 512], F32, tag="spair")
                for h in range(H_PER_CORE):
                    nc.tensor.matmul(
                        yp[:, 0, :],
                        lhsT=outn_sb[0:64, h, 128 * nt : 128 * (nt + 1)],
                        rhs=pj_sb[0:64, h, :],
                        start=(h == 0),
                        stop=(h == 3),
                    )
                yt = small.tile([128, 512], F32, tag="ytile")
                nc.vector.tensor_copy(yt[:, :], yp[:, 0, :])
                nc.sync.dma_start(y_d.ap()[128 * nt : 128 * (nt + 1), :], yt[:, :])

            # Piecewise emission schedule for the previous block's normalize +
            # projection, indexed by the current block's mt. st is a dict
            # carrying the pending block's tiles.
            def sched_pending(st, mt):
                nci, pair, heads, ot_ps, nsl = st["blk"]
                if mt == 1:
                    for hi in range(2):
                        rc = small.tile([128, 512], F32R, tag="rc")
                        with nc.allow_low_precision(reason="f32r holds full fp32 bits"):
                            nc.vector.reciprocal(rc[64:65, :], ot_ps[64:65, hi, :])
                        st[f"rc{hi}"] = rc
                elif mt in (5, 6):
                    hi = mt - 5
                    bc = psum.tile([128, 2, 512], F32, tag="spair")
                    nc.tensor.matmul(
                        bc[0:64, 0, :],
                        lhsT=ones_sb[64:65, 0:64],
                        rhs=st[f"rc{hi}"][64:65, :],
                        start=True,
                        stop=True,
                    )
                    bc_sb = small.tile([64, 512], F32, tag="bcsb")
                    nc.vector.tensor_copy(bc_sb[:, :], bc[0:64, 0, :])
                    st[f"bc{hi}"] = bc_sb
                elif mt == 8:
                    for hi, h in enumerate(heads):
                        nc.vector.tensor_tensor(
                            out=outn_sb[0:64, h, nsl],
                            in0=ot_ps[0:64, hi, :],
                            in1=st[f"bc{hi}"][:, :],
                            op=mybir.AluOpType.mult,
                        )
                elif mt in (10, 11, 12, 13) and pair == 1:
                    emit_y(4 * nci + (mt - 10))

            # ---- attention blocks ----
            # Before the first block only q (ot 0) and k (ot 2) exist; the
            # rest of qkv and all of v are injected just-in-time in block 0.
            for nt in range(NT512):
                emit_qkv_chunk(0, nt)
                emit_qkv_chunk(2, nt)

            pending = None
            for nci in range(NT512):
                nsl = slice(512 * nci, 512 * (nci + 1))
                for pair in range(2):
                    bi = 2 * nci + pair
                    heads = (2 * pair, 2 * pair + 1)
                    ot_ps = psum.tile([65, 2, 512], F32, tag="outT")
                    for mt in range(MT):
                        if bi == 0:
                            emit_v_chunk(mt)
                            if 4 <= mt < 8:
                                emit_qkv_chunk(1, mt - 4)
                            elif 8 <= mt < 12:
                                emit_qkv_chunk(3, mt - 8)
                        sp = psum.tile([128, 2, 512], F32, tag="spair")
                        for hi, h in enumerate(heads):
                            par = h % 2
                            otq = h // 2
                            psl = slice(64 * par, 64 * par + 64)
                            nc.tensor.matmul(
                                sp[:, hi, :],
                                lhsT=qk_sb[psl, 2 + otq, 128 * mt : 128 * (mt + 1)],
                                rhs=qk_sb[psl, otq, nsl],
                                start=True,
                                stop=True,
                            )
                        pt = work.tile([128, 2, 512], BF16, tag="ptile")
                        nc.scalar.activation(
                            pt[:, :, :],
                            sp[:, :, :],
                            mybir.ActivationFunctionType.Exp,
                            scale=float(EXP_SCALE),
                        )
                        for hi, h in enumerate(heads):
                            nc.tensor.matmul(
                                ot_ps[:, hi, :],
                                lhsT=v_sb[:, mt, h, :],
                                rhs=pt[:, hi, :],
                                start=(mt == 0),
                                stop=(mt == MT - 1),
                            )
                        if pending is not None:
                            sched_pending(pending, mt)
                    pending = {"blk": (nci, pair, heads, ot_ps, nsl)}
            # drain the last block's normalize + projection
            for mt in range(MT):
                sched_pending(pending, mt)

    nc.compile()
    return nc


_NC = None


def _get_nc():
    global _NC
    if _NC is None:
        _NC = build_nc()
    return _NC


def make_in_maps(x, qkv_w, qkv_b, proj_w):
    x = np.asarray(x, dtype=np.float32)
    qkv_w = np.asarray(qkv_w, dtype=np.float32)
    qkv_b = np.asarray(qkv_b, dtype=np.float32)
    proj_w = np.asarray(proj_w, dtype=np.float32)
    bf16 = ml_dtypes.bfloat16
    in_maps = []
    for core in range(8):
        b, g = divmod(core, 2)
        o0 = 256 * g
        wqk = np.concatenate([qkv_w[o0 : o0 + 256], qkv_w[512 + o0 : 512 + o0 + 256]], axis=0)
        bqk = np.concatenate([qkv_b[o0 : o0 + 256], qkv_b[512 + o0 : 512 + o0 + 256]], axis=0)
        wv = qkv_w[1024 + o0 : 1024 + o0 + 256]
        bv = qkv_b[1024 + o0 : 1024 + o0 + 256]
        pj = proj_w[:, o0 : o0 + 256]  # [512(co), 256(ci)]
        projt = np.ascontiguousarray(
            pj.T.reshape(4, 64, 512).transpose(1, 0, 2)
        ).astype(bf16)  # [64(d), 4(h), 512(co)]
        in_maps.append(
            {
                "xt": np.ascontiguousarray(x[b].T).astype(bf16),
                "wqk_t": np.ascontiguousarray(wqk.T).astype(bf16),
                "bqk": np.ascontiguousarray(bqk),
                "wv_t": np.ascontiguousarray(wv.T).astype(bf16),
                "bv_bcast": np.ascontiguousarray(np.tile(bv[None, :], (128, 1))),
                "projt": projt,
            }
        )
    return in_maps


def kernel(x, qkv_w, qkv_b, proj_w, proj_b, _trace=False, _trace_kwargs=None):
    nc = _get_nc()
    in_maps = make_in_maps(x, qkv_w, qkv_b, proj_w)
    kw = {}
    if _trace:
        kw = {"trace": True, **(_trace_kwargs or {})}
    res = run_bass_kernel_spmd(nc, in_maps, core_ids=list(range(8)), **kw)
    proj_b = np.asarray(proj_b, dtype=np.float32)
    y = np.empty((B, N, C), dtype=np.float32)
    for b in range(B):
        y[b] = res.results[2 * b]["y"] + res.results[2 * b + 1]["y"] + proj_b
    kernel.last_results = res
    return y
